# revision 4
# baseline (speedup 1.0000x reference)
"""Multi-head attention (B=2, S=2048, D=1024, H=16) on 8 TRN2 NeuronCores.

Sharding: core = (batch b, head-group g) with 2 batches x 4 groups of 4 heads.
Each core computes its group's QKV projections, attention, and a partial
output projection; the host sums the 4 partials per batch and adds the
bias constant (bv @ Wo.T + bo).

All matmuls run in float32r (TF32-like rounding, 1 cycle/row on the PE at
moving-dim >= 256, vs 4 cycles/row for exact fp32; measured ~1.3e-4 max rel
error on K=1024 matmuls).

Layout notes (per core):
  xT      [D, S]    transposed input (host-transposed), contraction dim on
                    partitions for the projection matmuls
  QT, KT  [JJ, S]   head-dim-major projections; scores S.T = KT_tile.T @ QT
                    so softmax rows (keys) land on partitions and the
                    reduction over keys happens in the P.T @ V matmul via an
                    appended ones-column on V (row 64 of the PV psum output
                    accumulates the softmax denominator)
  V'      [S, h, 65] natural-layout V with a ones column per head
  OT      [JJ, S]   normalized attention output, head-dim-major, feeding the
                    output projection as lhsT (out natural [S, D])
"""
import os
from contextlib import ExitStack

import numpy as np

# Problem constants (hardcoded per harness contract).
B, S, D, H = 2, 2048, 1024, 16
HD = D // H          # 64
N_CORES = 8
GROUPS = N_CORES // B    # 4
H_LOC = H // GROUPS      # 4 heads per core
JJ = H_LOC * HD          # 256
P = 128


def build_mha(s=S, d=D, h_loc=H_LOC, hd=HD, chunk=1024, nf=512):
    """Build the per-core Bass program. Returns the compiled Bacc object."""
    import concourse.bacc as bacc
    import concourse.tile as tile
    from concourse import mybir

    f32 = mybir.dt.float32
    f32r = mybir.dt.float32r
    Exp = mybir.ActivationFunctionType.Exp
    Ident = mybir.ActivationFunctionType.Identity

    jj = h_loc * hd
    hd1 = hd + 1
    ktd = d // P            # contraction tiles for projections
    njt = (jj + P - 1) // P  # QT/KT partition tiles
    st_n = s // P           # sequence tiles
    chunk = min(chunk, s)
    nf = min(nf, chunk)
    n_ch = s // chunk       # attention free-dim chunks
    nfc = chunk // nf       # matmul pieces per chunk
    ndo = (d + nf - 1) // nf  # out-proj free pieces

    nc = bacc.Bacc("TRN2", target_bir_lowering=False, debug=False)

    xq = nc.dram_tensor("xq", [d, s], f32, kind="ExternalInput").ap()
    xk = nc.dram_tensor("xk", [d, s], f32, kind="ExternalInput").ap()
    xv = nc.dram_tensor("xv", [d, s], f32, kind="ExternalInput").ap()
    wq = nc.dram_tensor("wq", [d, jj], f32, kind="ExternalInput").ap()
    wk = nc.dram_tensor("wk", [d, jj], f32, kind="ExternalInput").ap()
    wv = nc.dram_tensor("wv", [d, jj], f32, kind="ExternalInput").ap()
    wo = nc.dram_tensor("wo", [jj, d], f32, kind="ExternalInput").ap()
    bqp = nc.dram_tensor("bqp", [jj, 1], f32, kind="ExternalInput").ap()
    bkp = nc.dram_tensor("bkp", [jj, 1], f32, kind="ExternalInput").ap()
    out = nc.dram_tensor("out", [s, d], f32, kind="ExternalOutput").ap()

    with tile.TileContext(nc) as tc, ExitStack() as ctx:
        persist = ctx.enter_context(tc.tile_pool(name="persist", bufs=1))

        # ---- persistent SBUF tensors ----
        qt_sb = [persist.tile([P, s], f32r, name=f"qt{j}", tag=f"qt{j}") for j in range(njt)]
        kt_sb = [persist.tile([P, s], f32r, name=f"kt{j}", tag=f"kt{j}") for j in range(njt)]
        ot_sb = [persist.tile([P, s], f32r, name=f"ot{j}", tag=f"ot{j}") for j in range(njt)]
        v_sb = [persist.tile([P, h_loc, hd1], f32r, name=f"v{t}", tag=f"v{t}")
                for t in range(st_n)]
        wq_r = [persist.tile([P, jj], f32r, name=f"wqr{k}", tag=f"wqr{k}") for k in range(ktd)]
        wk_r = [persist.tile([P, jj], f32r, name=f"wkr{k}", tag=f"wkr{k}") for k in range(ktd)]
        wv_r = [persist.tile([P, jj], f32r, name=f"wvr{k}", tag=f"wvr{k}") for k in range(ktd)]
        wo_r = [persist.tile([P, d], f32r, name=f"wor{j}", tag=f"wor{j}") for j in range(njt)]
        bq_sb = persist.tile([P, njt], f32, name="bq_sb", tag="bq_sb")
        bk_sb = persist.tile([P, njt], f32, name="bk_sb", tag="bk_sb")
        ones_v = persist.tile([P, h_loc, 1], f32, name="ones_v", tag="ones_v")
        ones_h = persist.tile([1, hd], f32, name="ones_h", tag="ones_h")
        ones_hr = persist.tile([1, hd], f32r, name="ones_hr", tag="ones_hr")

        nc.vector.memset(ones_v[:], 1.0)
        nc.vector.memset(ones_h[:], 1.0)
        nc.vector.tensor_copy(ones_hr[:], ones_h[:])
        for j in range(njt):
            nc.sync.dma_start(bq_sb[:, j:j + 1], bqp[j * P:(j + 1) * P, :])
            nc.sync.dma_start(bk_sb[:, j:j + 1], bkp[j * P:(j + 1) * P, :])

        # ---- weights: DMA f32 staging -> DVE round to f32r ----
        with tc.tile_pool(name="wstage", bufs=3) as wstage:
            for k in range(ktd):
                for nm, dr, dst in (("q", wq, wq_r), ("k", wk, wk_r), ("v", wv, wv_r)):
                    wtmp = wstage.tile([P, jj], f32, name=f"w{nm}s{k}", tag="wst")
                    nc.sync.dma_start(wtmp[:], dr[k * P:(k + 1) * P, :])
                    nc.vector.tensor_copy(dst[k][:], wtmp[:])
            for j in range(njt):
                wtmp = wstage.tile([P, d], f32, name=f"wos{j}", tag="wost")
                nc.sync.dma_start(wtmp[:], wo[j * P:(j + 1) * P, :])
                nc.vector.tensor_copy(wo_r[j][:], wtmp[:])

        # ---- projections ----
        with tc.tile_pool(name="xpool", bufs=3) as xpool, \
             tc.tile_pool(name="xrpool", bufs=ktd) as xrpool, \
             tc.tile_pool(name="ppsum", bufs=3, space="PSUM") as ppsum:

            def load_xr(xdr):
                tiles = []
                for k in range(ktd):
                    xs = xpool.tile([P, s], f32, name=f"xs{k}", tag="xs")
                    nc.sync.dma_start(xs[:], xdr[k * P:(k + 1) * P, :])
                    xr = xrpool.tile([P, s], f32r, name=f"xr{k}", tag="xr")
                    nc.vector.tensor_copy(xr[:], xs[:])
                    tiles.append(xr)
                return tiles

            # K then Q (head-dim-major), each evicted via ScalarE with bias.
            pc = min(512, s)
            for nm, xdr, w_r, dst, bias_sb, scale in (
                ("k", xk, wk_r, kt_sb, bk_sb, 1.0),
                ("q", xq, wq_r, qt_sb, bq_sb, float(1.0 / np.sqrt(hd))),
            ):
                xr_t = load_xr(xdr)
                for j in range(njt):
                    for c in range(s // pc):
                        pp = ppsum.tile([P, pc], mybir.dt.float32,
                                        name=f"pp{nm}{j}_{c}", tag="pp")
                        for k in range(ktd):
                            nc.tensor.matmul(
                                pp[:], w_r[k][:, j * P:(j + 1) * P],
                                xr_t[k][:, c * pc:(c + 1) * pc],
                                start=(k == 0), stop=(k == ktd - 1))
                        nc.scalar.activation(
                            dst[j][:, c * pc:(c + 1) * pc], pp[:], Ident,
                            bias=bias_sb[:, j:j + 1], scale=scale)

            # V natural with ones column.
            xr_t = load_xr(xv)
            for t in range(st_n):
                pv = ppsum.tile([P, jj], mybir.dt.float32, name=f"pv{t}", tag="pv")
                for k in range(ktd):
                    nc.tensor.matmul(pv[:], xr_t[k][:, t * P:(t + 1) * P],
                                     wv_r[k][:], start=(k == 0), stop=(k == ktd - 1))
                nc.vector.tensor_copy(
                    v_sb[t][:, :, 0:hd],
                    pv[:].rearrange("p (h e) -> p h e", h=h_loc))
                nc.vector.tensor_copy(v_sb[t][:, :, hd:hd1], ones_v[:])

        # ---- attention ----
        with tc.tile_pool(name="spsum", bufs=2, space="PSUM") as spsum, \
             tc.tile_pool(name="opsum", bufs=2, space="PSUM") as opsum, \
             tc.tile_pool(name="ptpool", bufs=4) as ptpool, \
             tc.tile_pool(name="npool", bufs=2) as npool:
            for h in range(h_loc):
                jt = (h * hd) // P
                off = (h * hd) % P
                for c in range(n_ch):
                    otp = opsum.tile([hd1, chunk], mybir.dt.float32,
                                     name=f"otp{h}_{c}", tag="otp")
                    for t in range(st_n):
                        sp = spsum.tile([P, chunk], mybir.dt.float32,
                                        name=f"sp{h}_{c}_{t}", tag="sp")
                        for f in range(nfc):
                            nc.tensor.matmul(
                                sp[:, f * nf:(f + 1) * nf],
                                kt_sb[jt][off:off + hd, t * P:(t + 1) * P],
                                qt_sb[jt][off:off + hd,
                                          c * chunk + f * nf:c * chunk + (f + 1) * nf],
                                start=True, stop=True)
                        pt = ptpool.tile([P, chunk], f32r, name=f"pt{h}_{c}_{t}", tag="pt")
                        nc.scalar.activation(pt[:], sp[:], Exp)
                        for f in range(nfc):
                            nc.tensor.matmul(
                                otp[:, f * nf:(f + 1) * nf],
                                v_sb[t][:, h, :],
                                pt[:, f * nf:(f + 1) * nf],
                                start=(t == 0), stop=(t == st_n - 1))
                    # normalize: rows 0:hd scaled by 1/rowsum (row hd)
                    rs = npool.tile([1, chunk], f32, name=f"rs{h}_{c}", tag="rs")
                    nc.vector.tensor_copy(rs[:], otp[hd:hd1, :])
                    inv = npool.tile([1, chunk], f32, name=f"inv{h}_{c}", tag="inv")
                    nc.vector.reciprocal(inv[:], rs[:])
                    invr = npool.tile([1, chunk], f32r, name=f"invr{h}_{c}", tag="invr")
                    nc.vector.tensor_copy(invr[:], inv[:])
                    bp = spsum.tile([hd, chunk], mybir.dt.float32,
                                    name=f"bp{h}_{c}", tag="sp")
                    for f in range(nfc):
                        nc.tensor.matmul(bp[:, f * nf:(f + 1) * nf], ones_hr[:],
                                         invr[:, f * nf:(f + 1) * nf],
                                         start=True, stop=True)
                    bp_sb = npool.tile([hd, chunk], f32, name=f"bps{h}_{c}", tag="bps")
                    nc.vector.tensor_copy(bp_sb[:], bp[:])
                    nc.vector.tensor_mul(
                        ot_sb[jt][off:off + hd, c * chunk:(c + 1) * chunk],
                        otp[0:hd, :], bp_sb[:])

        # ---- output projection (natural layout) ----
        with tc.tile_pool(name="fpsum", bufs=2, space="PSUM") as fpsum, \
             tc.tile_pool(name="fout", bufs=2) as fout:
            for t in range(st_n):
                po = fpsum.tile([P, d], mybir.dt.float32, name=f"po{t}", tag="po")
                for nj in range(ndo):
                    for j in range(njt):
                        nc.tensor.matmul(
                            po[:, nj * nf:(nj + 1) * nf],
                            ot_sb[j][:, t * P:(t + 1) * P],
                            wo_r[j][:, nj * nf:(nj + 1) * nf],
                            start=(j == 0), stop=(j == njt - 1))
                ob = fout.tile([P, d], f32, name=f"ob{t}", tag="ob")
                nc.scalar.copy(ob[:], po[:])
                nc.sync.dma_start(out[t * P:(t + 1) * P, :], ob[:])

    nc.compile()
    return nc


_NC_CACHE = {}


def _get_nc():
    key = "full"
    if key not in _NC_CACHE:
        _NC_CACHE[key] = build_mha()
    return _NC_CACHE[key]


def build_in_maps(inputs):
    q = np.asarray(inputs["query"], np.float32)
    k = np.asarray(inputs.get("key_", inputs.get("key")), np.float32)
    v = np.asarray(inputs["value"], np.float32)
    Wq = np.asarray(inputs["Wq"], np.float32)
    Wk = np.asarray(inputs["Wk"], np.float32)
    Wv = np.asarray(inputs["Wv"], np.float32)
    Wo = np.asarray(inputs["Wo"], np.float32)
    bq = np.asarray(inputs["bq"], np.float32)
    bk = np.asarray(inputs["bk"], np.float32)

    sc = np.float32(1.0 / np.sqrt(HD))
    qT = [np.ascontiguousarray(q[b].T) for b in range(B)]
    kT = [np.ascontiguousarray(k[b].T) for b in range(B)]
    vT = [np.ascontiguousarray(v[b].T) for b in range(B)]
    WqT = np.ascontiguousarray(Wq.T)  # [d, j]
    WkT = np.ascontiguousarray(Wk.T)
    WvT = np.ascontiguousarray(Wv.T)

    in_maps = []
    for core in range(N_CORES):
        b, g = divmod(core, GROUPS)
        sl = slice(g * JJ, (g + 1) * JJ)
        in_maps.append({
            "xq": qT[b],
            "xk": kT[b],
            "xv": vT[b],
            "wq": np.ascontiguousarray(WqT[:, sl]),
            "wk": np.ascontiguousarray(WkT[:, sl]),
            "wv": np.ascontiguousarray(WvT[:, sl]),
            "wo": np.ascontiguousarray(Wo[:, sl].T),
            "bqp": np.ascontiguousarray((bq[sl] * sc)[:, None]),
            "bkp": np.ascontiguousarray(bk[sl][:, None]),
        })
    return in_maps


def combine_outputs(results, inputs):
    Wo = np.asarray(inputs["Wo"], np.float32)
    bv = np.asarray(inputs["bv"], np.float32)
    bo = np.asarray(inputs["bo"], np.float32)
    const = bv @ Wo.T + bo  # exact host-side bias correction
    outp = np.empty((B, S, D), np.float32)
    for b in range(B):
        acc = results[b * GROUPS]["out"].astype(np.float32).copy()
        for g in range(1, GROUPS):
            acc += results[b * GROUPS + g]["out"]
        outp[b] = acc + const[None, :]
    return outp


def kernel(**inputs):
    from concourse.bass_utils import run_bass_kernel_spmd

    nc = _get_nc()
    in_maps = build_in_maps(inputs)
    res = run_bass_kernel_spmd(nc, in_maps, list(range(N_CORES)))
    return combine_outputs(res.results, inputs)


# revision 5
# speedup vs baseline: 1.4069x; 1.4069x over previous
"""Multi-head attention (B=2, S=2048, D=1024, H=16) on 8 TRN2 NeuronCores.

Sharding: core = (batch b, head-group g): 2 batches x 4 groups of 4 heads.
Each core computes its group's QKV projections, attention, and a partial
output projection; the host sums the 4 partials per batch and adds the
exact bias constant (bv @ Wo.T + bo). bq/bk are applied on device.

Matmul dtype is configurable:
  bf16: operands stored/shipped as bfloat16, 1 PE cycle/row + fast weight
        load; fp32 PSUM accumulation. ~3e-3 max rel error.
  f32r: fp32 data rounded to the PE's TF32-like fast format, 2 cycles/row.
        ~5e-4 max rel error.
The softmax normalization chain stays in f32/f32r in either mode so the
denominator carries no bf16 error.

Per-core layout:
  xT [D, S] host-transposed inputs; QT/KT [JJ, S] head-dim-major so scores
  come out keys-on-partitions (S.T tiles) and the key-axis softmax reduction
  happens inside the P.T @ V' matmul via a ones-column appended to V'
  (PSUM row 64 of the PV output accumulates the softmax denominator).
  V' stationaries are padded to 128 columns to keep fast weight loads.
  OT [JJ, S] normalized attention output feeds the output projection as
  lhsT, giving the partial output in natural [S, D] layout.
"""
from contextlib import ExitStack

import numpy as np

# Problem constants (hardcoded per harness contract).
B, S, D, H = 2, 2048, 1024, 16
HD = D // H          # 64
N_CORES = 8
GROUPS = N_CORES // B    # 4
H_LOC = H // GROUPS      # 4 heads per core
JJ = H_LOC * HD          # 256
P = 128

MM_DT = "bf16"  # "bf16" | "f32r"


def build_mha(s=S, d=D, h_loc=H_LOC, hd=HD, chunk=1024, nf=512, mm_dt=MM_DT):
    """Build + compile the per-core Bass program."""
    import concourse.bacc as bacc
    import concourse.tile as tile
    from concourse import mybir

    f32 = mybir.dt.float32
    f32r = mybir.dt.float32r
    bf16 = mybir.dt.bfloat16
    mdt = bf16 if mm_dt == "bf16" else f32r
    in_dt = bf16 if mm_dt == "bf16" else f32  # DRAM dtype of x / weights
    Exp = mybir.ActivationFunctionType.Exp
    Ident = mybir.ActivationFunctionType.Identity

    jj = h_loc * hd
    hd1 = hd + 1
    ktd = d // P
    njt = (jj + P - 1) // P
    st_n = s // P
    chunk = min(chunk, s)
    nf = min(nf, chunk)
    n_ch = s // chunk
    nfc = chunk // nf
    ndo = (d + nf - 1) // nf
    pc = min(512, s)

    nc = bacc.Bacc("TRN2", target_bir_lowering=False, debug=False)

    xq = nc.dram_tensor("xq", [d, s], in_dt, kind="ExternalInput").ap()
    xk = nc.dram_tensor("xk", [d, s], in_dt, kind="ExternalInput").ap()
    xv = nc.dram_tensor("xv", [d, s], in_dt, kind="ExternalInput").ap()
    wq = nc.dram_tensor("wq", [d, jj], in_dt, kind="ExternalInput").ap()
    wk = nc.dram_tensor("wk", [d, jj], in_dt, kind="ExternalInput").ap()
    wv = nc.dram_tensor("wv", [d, jj], in_dt, kind="ExternalInput").ap()
    wo = nc.dram_tensor("wo", [jj, d], in_dt, kind="ExternalInput").ap()
    bqp = nc.dram_tensor("bqp", [jj, 1], f32, kind="ExternalInput").ap()
    bkp = nc.dram_tensor("bkp", [jj, 1], f32, kind="ExternalInput").ap()
    out = nc.dram_tensor("out", [s, d], f32, kind="ExternalOutput").ap()

    with tile.TileContext(nc) as tc, ExitStack() as ctx:
        persist = ctx.enter_context(tc.tile_pool(name="persist", bufs=1))

        qt_sb = [persist.tile([P, s], mdt, name=f"qt{j}", tag=f"qt{j}") for j in range(njt)]
        kt_sb = [persist.tile([P, s], mdt, name=f"kt{j}", tag=f"kt{j}") for j in range(njt)]
        ot_sb = [persist.tile([P, s], mdt, name=f"ot{j}", tag=f"ot{j}") for j in range(njt)]
        # padded per-(seq-tile, head) PV stationaries: [V_h | ones | zeros]
        v_sb = [[persist.tile([P, P], mdt, name=f"v{t}_{h}", tag=f"v{t}_{h}")
                 for h in range(h_loc)] for t in range(st_n)]
        wq_r = [persist.tile([P, jj], mdt, name=f"wqr{k}", tag=f"wqr{k}") for k in range(ktd)]
        wk_r = [persist.tile([P, jj], mdt, name=f"wkr{k}", tag=f"wkr{k}") for k in range(ktd)]
        wv_r = [persist.tile([P, jj], mdt, name=f"wvr{k}", tag=f"wvr{k}") for k in range(ktd)]
        wo_r = [persist.tile([P, d], mdt, name=f"wor{j}", tag=f"wor{j}") for j in range(njt)]
        bq_sb = persist.tile([P, njt], f32, name="bq_sb", tag="bq_sb")
        bk_sb = persist.tile([P, njt], f32, name="bk_sb", tag="bk_sb")
        ones_v = persist.tile([P, 1], f32, name="ones_v", tag="ones_v")
        ones_h = persist.tile([1, hd], f32, name="ones_h", tag="ones_h")
        ones_hr = persist.tile([1, hd], f32r, name="ones_hr", tag="ones_hr")

        nc.vector.memset(ones_v[:], 1.0)
        nc.vector.memset(ones_h[:], 1.0)
        nc.vector.tensor_copy(ones_hr[:], ones_h[:])
        for j in range(njt):
            nc.sync.dma_start(bq_sb[:, j:j + 1], bqp[j * P:(j + 1) * P, :])
            nc.sync.dma_start(bk_sb[:, j:j + 1], bkp[j * P:(j + 1) * P, :])

        # ---- weights ----
        if mm_dt == "bf16":
            for k in range(ktd):
                nc.sync.dma_start(wq_r[k][:], wq[k * P:(k + 1) * P, :])
                nc.sync.dma_start(wk_r[k][:], wk[k * P:(k + 1) * P, :])
                nc.sync.dma_start(wv_r[k][:], wv[k * P:(k + 1) * P, :])
            for j in range(njt):
                nc.sync.dma_start(wo_r[j][:], wo[j * P:(j + 1) * P, :])
        else:
            with tc.tile_pool(name="wstage", bufs=3) as wstage:
                for k in range(ktd):
                    for nm, dr, dst in (("q", wq, wq_r), ("k", wk, wk_r), ("v", wv, wv_r)):
                        wtmp = wstage.tile([P, jj], f32, name=f"w{nm}s{k}", tag="wst")
                        nc.sync.dma_start(wtmp[:], dr[k * P:(k + 1) * P, :])
                        nc.vector.tensor_copy(dst[k][:], wtmp[:])
                for j in range(njt):
                    wtmp = wstage.tile([P, d], f32, name=f"wos{j}", tag="wost")
                    nc.sync.dma_start(wtmp[:], wo[j * P:(j + 1) * P, :])
                    nc.vector.tensor_copy(wo_r[j][:], wtmp[:])

        # ---- projections ----
        with tc.tile_pool(name="xpool", bufs=3) as xpool, \
             tc.tile_pool(name="xrpool", bufs=ktd) as xrpool, \
             tc.tile_pool(name="ppsum", bufs=3, space="PSUM") as ppsum:

            def load_xr(xdr):
                tiles = []
                for k in range(ktd):
                    if mm_dt == "bf16":
                        xr = xrpool.tile([P, s], mdt, name=f"xr{k}", tag="xr")
                        nc.sync.dma_start(xr[:], xdr[k * P:(k + 1) * P, :])
                    else:
                        xs = xpool.tile([P, s], f32, name=f"xs{k}", tag="xs")
                        nc.sync.dma_start(xs[:], xdr[k * P:(k + 1) * P, :])
                        xr = xrpool.tile([P, s], mdt, name=f"xr{k}", tag="xr")
                        nc.vector.tensor_copy(xr[:], xs[:])
                    tiles.append(xr)
                return tiles

            for nm, xdr, w_r, dst, bias_sb, scale in (
                ("k", xk, wk_r, kt_sb, bk_sb, 1.0),
                ("q", xq, wq_r, qt_sb, bq_sb, float(1.0 / np.sqrt(hd))),
            ):
                xr_t = load_xr(xdr)
                for j in range(njt):
                    for c in range(s // pc):
                        pp = ppsum.tile([P, pc], f32, name=f"pp{nm}{j}_{c}", tag="pp")
                        for k in range(ktd):
                            nc.tensor.matmul(
                                pp[:], w_r[k][:, j * P:(j + 1) * P],
                                xr_t[k][:, c * pc:(c + 1) * pc],
                                start=(k == 0), stop=(k == ktd - 1))
                        nc.scalar.activation(
                            dst[j][:, c * pc:(c + 1) * pc], pp[:], Ident,
                            bias=bias_sb[:, j:j + 1], scale=scale)

            # V' padded stationaries
            xr_t = load_xr(xv)
            for t in range(st_n):
                pv = ppsum.tile([P, jj], f32, name=f"pv{t}", tag="pv")
                for k in range(ktd):
                    nc.tensor.matmul(pv[:], xr_t[k][:, t * P:(t + 1) * P],
                                     wv_r[k][:], start=(k == 0), stop=(k == ktd - 1))
                for h in range(h_loc):
                    vt = v_sb[t][h]
                    nc.vector.tensor_copy(vt[:, 0:hd], pv[:, h * hd:(h + 1) * hd])
                    nc.vector.tensor_copy(vt[:, hd:hd1], ones_v[:])
                    if hd1 < P:
                        nc.gpsimd.memset(vt[:, hd1:P], 0.0)

        # ---- attention ----
        with tc.tile_pool(name="spsum", bufs=2, space="PSUM") as spsum, \
             tc.tile_pool(name="opsum", bufs=2, space="PSUM") as opsum, \
             tc.tile_pool(name="ptpool", bufs=4) as ptpool, \
             tc.tile_pool(name="npool", bufs=2) as npool:
            for h in range(h_loc):
                jt = (h * hd) // P
                off = (h * hd) % P
                for c in range(n_ch):
                    otp = opsum.tile([P, chunk], f32, name=f"otp{h}_{c}", tag="otp")
                    for t in range(st_n):
                        sp = spsum.tile([P, chunk], f32, name=f"sp{h}_{c}_{t}", tag="sp")
                        for f in range(nfc):
                            nc.tensor.matmul(
                                sp[:, f * nf:(f + 1) * nf],
                                kt_sb[jt][off:off + hd, t * P:(t + 1) * P],
                                qt_sb[jt][off:off + hd,
                                          c * chunk + f * nf:c * chunk + (f + 1) * nf],
                                start=True, stop=True)
                        pt = ptpool.tile([P, chunk], mdt, name=f"pt{h}_{c}_{t}", tag="pt")
                        nc.scalar.activation(pt[:], sp[:], Exp)
                        for f in range(nfc):
                            nc.tensor.matmul(
                                otp[:, f * nf:(f + 1) * nf],
                                v_sb[t][h][:],
                                pt[:, f * nf:(f + 1) * nf],
                                start=(t == 0), stop=(t == st_n - 1))
                    # normalize rows 0:hd by row hd (the softmax denominator)
                    rs_r = npool.tile([1, chunk], f32r, name=f"rs{h}_{c}", tag="rs")
                    nc.scalar.activation(rs_r[:], otp[hd:hd1, :],
                                         mybir.ActivationFunctionType.Copy)
                    bp = spsum.tile([hd, chunk], f32, name=f"bp{h}_{c}", tag="sp")
                    for f in range(nfc):
                        nc.tensor.matmul(bp[:, f * nf:(f + 1) * nf], ones_hr[:],
                                         rs_r[:, f * nf:(f + 1) * nf],
                                         start=True, stop=True)
                    binv = npool.tile([hd, chunk], f32, name=f"binv{h}_{c}", tag="binv")
                    nc.vector.reciprocal(binv[:], bp[:])
                    nc.vector.tensor_mul(
                        ot_sb[jt][off:off + hd, c * chunk:(c + 1) * chunk],
                        otp[0:hd, :], binv[:])

        # ---- output projection (natural layout) ----
        with tc.tile_pool(name="fpsum", bufs=2, space="PSUM") as fpsum, \
             tc.tile_pool(name="fout", bufs=2) as fout:
            for t in range(st_n):
                po = fpsum.tile([P, d], f32, name=f"po{t}", tag="po")
                for njx in range(ndo):
                    for j in range(njt):
                        nc.tensor.matmul(
                            po[:, njx * nf:(njx + 1) * nf],
                            ot_sb[j][:, t * P:(t + 1) * P],
                            wo_r[j][:, njx * nf:(njx + 1) * nf],
                            start=(j == 0), stop=(j == njt - 1))
                ob = fout.tile([P, d], f32, name=f"ob{t}", tag="ob")
                nc.scalar.copy(ob[:], po[:])
                nc.sync.dma_start(out[t * P:(t + 1) * P, :], ob[:])

    nc.compile()
    return nc


_NC_CACHE = {}


def _get_nc():
    key = MM_DT
    if key not in _NC_CACHE:
        _NC_CACHE[key] = build_mha(mm_dt=key)
    return _NC_CACHE[key]


def build_in_maps(inputs, mm_dt=MM_DT):
    import ml_dtypes
    xdt = ml_dtypes.bfloat16 if mm_dt == "bf16" else np.float32

    q = np.asarray(inputs["query"], np.float32)
    k = np.asarray(inputs.get("key_", inputs.get("key")), np.float32)
    v = np.asarray(inputs["value"], np.float32)
    Wq = np.asarray(inputs["Wq"], np.float32)
    Wk = np.asarray(inputs["Wk"], np.float32)
    Wv = np.asarray(inputs["Wv"], np.float32)
    Wo = np.asarray(inputs["Wo"], np.float32)
    bq = np.asarray(inputs["bq"], np.float32)
    bk = np.asarray(inputs["bk"], np.float32)

    sc = np.float32(1.0 / np.sqrt(HD))
    qT = [np.ascontiguousarray(q[b].T).astype(xdt) for b in range(B)]
    kT = [np.ascontiguousarray(k[b].T).astype(xdt) for b in range(B)]
    vT = [np.ascontiguousarray(v[b].T).astype(xdt) for b in range(B)]
    WqT = np.ascontiguousarray(Wq.T)
    WkT = np.ascontiguousarray(Wk.T)
    WvT = np.ascontiguousarray(Wv.T)

    in_maps = []
    for core in range(N_CORES):
        b, g = divmod(core, GROUPS)
        sl = slice(g * JJ, (g + 1) * JJ)
        in_maps.append({
            "xq": qT[b],
            "xk": kT[b],
            "xv": vT[b],
            "wq": np.ascontiguousarray(WqT[:, sl]).astype(xdt),
            "wk": np.ascontiguousarray(WkT[:, sl]).astype(xdt),
            "wv": np.ascontiguousarray(WvT[:, sl]).astype(xdt),
            "wo": np.ascontiguousarray(Wo[:, sl].T).astype(xdt),
            "bqp": np.ascontiguousarray((bq[sl] * sc)[:, None]),
            "bkp": np.ascontiguousarray(bk[sl][:, None]),
        })
    return in_maps


def combine_outputs(results, inputs):
    Wo = np.asarray(inputs["Wo"], np.float32)
    bv = np.asarray(inputs["bv"], np.float32)
    bo = np.asarray(inputs["bo"], np.float32)
    const = bv @ Wo.T + bo  # exact host-side bias correction
    outp = np.empty((B, S, D), np.float32)
    for b in range(B):
        acc = results[b * GROUPS]["out"].astype(np.float32).copy()
        for g in range(1, GROUPS):
            acc += results[b * GROUPS + g]["out"]
        outp[b] = acc + const[None, :]
    return outp


def kernel(**inputs):
    from concourse.bass_utils import run_bass_kernel_spmd

    nc = _get_nc()
    in_maps = build_in_maps(inputs)
    res = run_bass_kernel_spmd(nc, in_maps, list(range(N_CORES)))
    return combine_outputs(res.results, inputs)


# revision 9
# speedup vs baseline: 1.4211x; 1.0101x over previous
"""Multi-head attention (B=2, S=2048, D=1024, H=16) on 8 TRN2 NeuronCores.

Sharding: core = (batch b, head-group g): 2 batches x 4 groups of 4 heads.
Each core computes its group's QKV projections, attention, and a partial
output projection; the host sums the 4 partials per batch and adds the
exact bias constant (bv @ Wo.T + bo). bq/bk are applied on device.

Matmul dtype is configurable:
  bf16: operands stored/shipped as bfloat16, 1 PE cycle/row + fast weight
        load; fp32 PSUM accumulation. ~3e-3 max rel error.
  f32r: fp32 data rounded to the PE's TF32-like fast format, 2 cycles/row.
        ~5e-4 max rel error.
The softmax normalization chain stays in f32/f32r in either mode so the
denominator carries no bf16 error.

Per-core layout:
  xT [D, S] host-transposed inputs; QT/KT [JJ, S] head-dim-major so scores
  come out keys-on-partitions (S.T tiles) and the key-axis softmax reduction
  happens inside the P.T @ V' matmul via a ones-column appended to V'
  (PSUM row 64 of the PV output accumulates the softmax denominator).
  V' stationaries are padded to 128 columns to keep fast weight loads.
  OT [JJ, S] normalized attention output feeds the output projection as
  lhsT, giving the partial output in natural [S, D] layout.
"""
from contextlib import ExitStack

import numpy as np

# Problem constants (hardcoded per harness contract).
B, S, D, H = 2, 2048, 1024, 16
HD = D // H          # 64
N_CORES = 8
GROUPS = N_CORES // B    # 4
H_LOC = H // GROUPS      # 4 heads per core
JJ = H_LOC * HD          # 256
P = 128

MM_DT = "bf16"  # "bf16" | "f32r"


def build_mha(s=S, d=D, h_loc=H_LOC, hd=HD, chunk=1024, nf=512, mm_dt=MM_DT):
    """Build + compile the per-core Bass program."""
    import concourse.bacc as bacc
    import concourse.tile as tile
    from concourse import mybir

    f32 = mybir.dt.float32
    f32r = mybir.dt.float32r
    bf16 = mybir.dt.bfloat16
    mdt = bf16 if mm_dt == "bf16" else f32r
    in_dt = bf16 if mm_dt == "bf16" else f32  # DRAM dtype of x / weights
    Exp = mybir.ActivationFunctionType.Exp
    Ident = mybir.ActivationFunctionType.Identity

    jj = h_loc * hd
    hd1 = hd + 1
    ktd = d // P
    njt = (jj + P - 1) // P
    st_n = s // P
    chunk = min(chunk, s)
    nf = min(nf, chunk)
    n_ch = s // chunk
    nfc = chunk // nf
    ndo = (d + nf - 1) // nf
    pc = min(512, s)

    nc = bacc.Bacc("TRN2", target_bir_lowering=False, debug=False)

    xq = nc.dram_tensor("xq", [d, s], in_dt, kind="ExternalInput").ap()
    xk = nc.dram_tensor("xk", [d, s], in_dt, kind="ExternalInput").ap()
    xv = nc.dram_tensor("xv", [d, s], in_dt, kind="ExternalInput").ap()
    wq = nc.dram_tensor("wq", [d, jj], in_dt, kind="ExternalInput").ap()
    wk = nc.dram_tensor("wk", [d, jj], in_dt, kind="ExternalInput").ap()
    wv = nc.dram_tensor("wv", [d, jj], in_dt, kind="ExternalInput").ap()
    wo = nc.dram_tensor("wo", [jj, d], in_dt, kind="ExternalInput").ap()
    bqp = nc.dram_tensor("bqp", [jj, 1], f32, kind="ExternalInput").ap()
    bkp = nc.dram_tensor("bkp", [jj, 1], f32, kind="ExternalInput").ap()
    out = nc.dram_tensor("out", [s, d], f32, kind="ExternalOutput").ap()

    with tile.TileContext(nc) as tc, ExitStack() as ctx:
        persist = ctx.enter_context(tc.tile_pool(name="persist", bufs=1))

        qt_sb = [persist.tile([P, s], mdt, name=f"qt{j}", tag=f"qt{j}") for j in range(njt)]
        kt_sb = [persist.tile([P, s], mdt, name=f"kt{j}", tag=f"kt{j}") for j in range(njt)]
        ot_sb = [persist.tile([P, s], mdt, name=f"ot{j}", tag=f"ot{j}") for j in range(njt)]
        # padded per-(seq-tile, head) PV stationaries: [V_h | ones | zeros]
        v_sb = [[persist.tile([P, P], mdt, name=f"v{t}_{h}", tag=f"v{t}_{h}")
                 for h in range(h_loc)] for t in range(st_n)]
        wq_r = [persist.tile([P, jj], mdt, name=f"wqr{k}", tag=f"wqr{k}") for k in range(ktd)]
        wk_r = [persist.tile([P, jj], mdt, name=f"wkr{k}", tag=f"wkr{k}") for k in range(ktd)]
        wv_r = [persist.tile([P, jj], mdt, name=f"wvr{k}", tag=f"wvr{k}") for k in range(ktd)]
        wo_r = [persist.tile([P, d], mdt, name=f"wor{j}", tag=f"wor{j}") for j in range(njt)]
        bq_sb = persist.tile([P, njt], f32, name="bq_sb", tag="bq_sb")
        bk_sb = persist.tile([P, njt], f32, name="bk_sb", tag="bk_sb")
        ones_v = persist.tile([P, 1], f32, name="ones_v", tag="ones_v")
        ones_h = persist.tile([1, hd], f32, name="ones_h", tag="ones_h")
        ones_hr = persist.tile([1, hd], f32r, name="ones_hr", tag="ones_hr")

        nc.vector.memset(ones_v[:], 1.0)
        nc.vector.memset(ones_h[:], 1.0)
        nc.vector.tensor_copy(ones_hr[:], ones_h[:])
        for j in range(njt):
            nc.sync.dma_start(bq_sb[:, j:j + 1], bqp[j * P:(j + 1) * P, :])
            nc.sync.dma_start(bk_sb[:, j:j + 1], bkp[j * P:(j + 1) * P, :])

        # ---- weights ----
        if mm_dt == "bf16":
            for k in range(ktd):
                nc.sync.dma_start(wq_r[k][:], wq[k * P:(k + 1) * P, :])
                nc.sync.dma_start(wk_r[k][:], wk[k * P:(k + 1) * P, :])
                nc.sync.dma_start(wv_r[k][:], wv[k * P:(k + 1) * P, :])
            for j in range(njt):
                nc.sync.dma_start(wo_r[j][:], wo[j * P:(j + 1) * P, :])
        else:
            with tc.tile_pool(name="wstage", bufs=3) as wstage:
                for k in range(ktd):
                    for nm, dr, dst in (("q", wq, wq_r), ("k", wk, wk_r), ("v", wv, wv_r)):
                        wtmp = wstage.tile([P, jj], f32, name=f"w{nm}s{k}", tag="wst")
                        nc.sync.dma_start(wtmp[:], dr[k * P:(k + 1) * P, :])
                        nc.vector.tensor_copy(dst[k][:], wtmp[:])
                for j in range(njt):
                    wtmp = wstage.tile([P, d], f32, name=f"wos{j}", tag="wost")
                    nc.sync.dma_start(wtmp[:], wo[j * P:(j + 1) * P, :])
                    nc.vector.tensor_copy(wo_r[j][:], wtmp[:])

        # ---- projections ----
        with tc.tile_pool(name="xpool", bufs=3) as xpool, \
             tc.tile_pool(name="xrpool", bufs=ktd) as xrpool, \
             tc.tile_pool(name="ppsum", bufs=3, space="PSUM") as ppsum:

            def load_xr(xdr):
                tiles = []
                for k in range(ktd):
                    if mm_dt == "bf16":
                        xr = xrpool.tile([P, s], mdt, name=f"xr{k}", tag="xr")
                        nc.sync.dma_start(xr[:], xdr[k * P:(k + 1) * P, :])
                    else:
                        xs = xpool.tile([P, s], f32, name=f"xs{k}", tag="xs")
                        nc.sync.dma_start(xs[:], xdr[k * P:(k + 1) * P, :])
                        xr = xrpool.tile([P, s], mdt, name=f"xr{k}", tag="xr")
                        nc.vector.tensor_copy(xr[:], xs[:])
                    tiles.append(xr)
                return tiles

            for nm, xdr, w_r, dst, bias_sb, scale in (
                ("k", xk, wk_r, kt_sb, bk_sb, 1.0),
                ("q", xq, wq_r, qt_sb, bq_sb, float(1.0 / np.sqrt(hd))),
            ):
                xr_t = load_xr(xdr)
                ncp = s // pc
                for j in range(njt):
                    # k-outer / c-inner so each weight stationary load serves
                    # ncp moving streams
                    pps = [ppsum.tile([P, pc], f32, name=f"pp{nm}{j}_{c}", tag="pp",
                                      bufs=ncp + 1)
                           for c in range(ncp)]
                    for k in range(ktd):
                        for c in range(ncp):
                            nc.tensor.matmul(
                                pps[c][:], w_r[k][:, j * P:(j + 1) * P],
                                xr_t[k][:, c * pc:(c + 1) * pc],
                                start=(k == 0), stop=(k == ktd - 1))
                    for c in range(ncp):
                        nc.scalar.activation(
                            dst[j][:, c * pc:(c + 1) * pc], pps[c][:], Ident,
                            bias=bias_sb[:, j:j + 1], scale=scale)

            # V' padded stationaries
            xr_t = load_xr(xv)
            for t in range(st_n):
                pv = ppsum.tile([P, jj], f32, name=f"pv{t}", tag="pv", bufs=3)
                for k in range(ktd):
                    nc.tensor.matmul(pv[:], xr_t[k][:, t * P:(t + 1) * P],
                                     wv_r[k][:], start=(k == 0), stop=(k == ktd - 1))
                for h in range(h_loc):
                    vt = v_sb[t][h]
                    nc.vector.tensor_copy(vt[:, 0:hd], pv[:, h * hd:(h + 1) * hd])
                    nc.vector.tensor_copy(vt[:, hd:hd1], ones_v[:])
                    if hd1 < P:
                        nc.gpsimd.memset(vt[:, hd1:P], 0.0)

        # ---- attention ----
        # Per head, two passes over the full sequence:
        #   pass 1: scores.T tiles (one KT stationary load per seq-tile, s/nf
        #           moving streams) -> exp over [128, s] -> PT tiles
        #   pass 2: PV accumulation (one V' stationary load per seq-tile,
        #           s/nf moving streams) -> [128, s] psum, row hd = denominators
        # PSUM: sp [128,s] (s/512 banks) + otp [128,s] -> 8 banks total.
        pt_bufs = st_n + 4
        with tc.tile_pool(name="spsum", bufs=1, space="PSUM") as spsum, \
             tc.tile_pool(name="opsum", bufs=1, space="PSUM") as opsum, \
             tc.tile_pool(name="ptpool", bufs=pt_bufs) as ptpool, \
             tc.tile_pool(name="npool", bufs=2) as npool:
            nff = s // nf
            for h in range(h_loc):
                jt = (h * hd) // P
                off = (h * hd) % P
                pts = []
                for t in range(st_n):
                    sp = spsum.tile([P, s], f32, name=f"sp{h}_{t}", tag="sp")
                    for f in range(nff):
                        nc.tensor.matmul(
                            sp[:, f * nf:(f + 1) * nf],
                            kt_sb[jt][off:off + hd, t * P:(t + 1) * P],
                            qt_sb[jt][off:off + hd, f * nf:(f + 1) * nf],
                            start=True, stop=True)
                    pt = ptpool.tile([P, s], mdt, name=f"pt{h}_{t}", tag="pt")
                    nc.scalar.activation(pt[:], sp[:], Exp)
                    pts.append(pt)
                otp = opsum.tile([P, s], f32, name=f"otp{h}", tag="otp")
                for t in range(st_n):
                    for f in range(nff):
                        nc.tensor.matmul(
                            otp[:, f * nf:(f + 1) * nf],
                            v_sb[t][h][:],
                            pts[t][:, f * nf:(f + 1) * nf],
                            start=(t == 0), stop=(t == st_n - 1))
                # normalize rows 0:hd by row hd (the softmax denominator)
                rs_r = npool.tile([1, s], f32r, name=f"rs{h}", tag="rs")
                nc.scalar.activation(rs_r[:], otp[hd:hd1, :],
                                     mybir.ActivationFunctionType.Copy)
                bp = spsum.tile([hd, s], f32, name=f"bp{h}", tag="sp")
                for f in range(nff):
                    nc.tensor.matmul(bp[:, f * nf:(f + 1) * nf], ones_hr[:],
                                     rs_r[:, f * nf:(f + 1) * nf],
                                     start=True, stop=True)
                binv = npool.tile([hd, s], f32, name=f"binv{h}", tag="binv")
                nc.vector.reciprocal(binv[:], bp[:])
                nc.vector.tensor_mul(
                    ot_sb[jt][off:off + hd, :], otp[0:hd, :], binv[:])

        # ---- output projection (natural layout) ----
        with tc.tile_pool(name="fpsum", bufs=2, space="PSUM") as fpsum, \
             tc.tile_pool(name="fout", bufs=2) as fout:
            for t in range(st_n):
                po = fpsum.tile([P, d], f32, name=f"po{t}", tag="po")
                for njx in range(ndo):
                    for j in range(njt):
                        nc.tensor.matmul(
                            po[:, njx * nf:(njx + 1) * nf],
                            ot_sb[j][:, t * P:(t + 1) * P],
                            wo_r[j][:, njx * nf:(njx + 1) * nf],
                            start=(j == 0), stop=(j == njt - 1))
                ob = fout.tile([P, d], f32, name=f"ob{t}", tag="ob")
                nc.scalar.copy(ob[:], po[:])
                nc.sync.dma_start(out[t * P:(t + 1) * P, :], ob[:])

    nc.compile()
    return nc


_NC_CACHE = {}


def _get_nc():
    key = MM_DT
    if key not in _NC_CACHE:
        _NC_CACHE[key] = build_mha(mm_dt=key)
    return _NC_CACHE[key]


def build_in_maps(inputs, mm_dt=MM_DT):
    import ml_dtypes
    xdt = ml_dtypes.bfloat16 if mm_dt == "bf16" else np.float32

    q = np.asarray(inputs["query"], np.float32)
    k = np.asarray(inputs.get("key_", inputs.get("key")), np.float32)
    v = np.asarray(inputs["value"], np.float32)
    Wq = np.asarray(inputs["Wq"], np.float32)
    Wk = np.asarray(inputs["Wk"], np.float32)
    Wv = np.asarray(inputs["Wv"], np.float32)
    Wo = np.asarray(inputs["Wo"], np.float32)
    bq = np.asarray(inputs["bq"], np.float32)
    bk = np.asarray(inputs["bk"], np.float32)

    sc = np.float32(1.0 / np.sqrt(HD))
    qT = [np.ascontiguousarray(q[b].T).astype(xdt) for b in range(B)]
    kT = [np.ascontiguousarray(k[b].T).astype(xdt) for b in range(B)]
    vT = [np.ascontiguousarray(v[b].T).astype(xdt) for b in range(B)]
    WqT = np.ascontiguousarray(Wq.T)
    WkT = np.ascontiguousarray(Wk.T)
    WvT = np.ascontiguousarray(Wv.T)

    in_maps = []
    for core in range(N_CORES):
        b, g = divmod(core, GROUPS)
        sl = slice(g * JJ, (g + 1) * JJ)
        in_maps.append({
            "xq": qT[b],
            "xk": kT[b],
            "xv": vT[b],
            "wq": np.ascontiguousarray(WqT[:, sl]).astype(xdt),
            "wk": np.ascontiguousarray(WkT[:, sl]).astype(xdt),
            "wv": np.ascontiguousarray(WvT[:, sl]).astype(xdt),
            "wo": np.ascontiguousarray(Wo[:, sl].T).astype(xdt),
            "bqp": np.ascontiguousarray((bq[sl] * sc)[:, None]),
            "bkp": np.ascontiguousarray(bk[sl][:, None]),
        })
    return in_maps


def combine_outputs(results, inputs):
    Wo = np.asarray(inputs["Wo"], np.float32)
    bv = np.asarray(inputs["bv"], np.float32)
    bo = np.asarray(inputs["bo"], np.float32)
    const = bv @ Wo.T + bo  # exact host-side bias correction
    outp = np.empty((B, S, D), np.float32)
    for b in range(B):
        acc = results[b * GROUPS]["out"].astype(np.float32).copy()
        for g in range(1, GROUPS):
            acc += results[b * GROUPS + g]["out"]
        outp[b] = acc + const[None, :]
    return outp


def kernel(**inputs):
    from concourse.bass_utils import run_bass_kernel_spmd

    nc = _get_nc()
    in_maps = build_in_maps(inputs)
    res = run_bass_kernel_spmd(nc, in_maps, list(range(N_CORES)))
    return combine_outputs(res.results, inputs)


# revision 11
# speedup vs baseline: 1.8794x; 1.3226x over previous
"""Multi-head attention (B=2, S=2048, D=1024, H=16) on 8 TRN2 NeuronCores.

Sharding: core = (batch b, head-group g): 2 batches x 4 groups of 4 heads.
Each core computes its group's QKV projections, attention, and a partial
output projection; the host sums the 4 partials per batch and adds the
exact bias constant (bv @ Wo.T + bo). bq/bk are applied on device.

Matmul dtype is configurable:
  bf16: operands stored/shipped as bfloat16, 1 PE cycle/row + fast weight
        load; fp32 PSUM accumulation. ~3e-3 max rel error.
  f32r: fp32 data rounded to the PE's TF32-like fast format, 2 cycles/row.
        ~5e-4 max rel error.
The softmax normalization chain stays in f32/f32r in either mode so the
denominator carries no bf16 error.

Per-core layout:
  xT [D, S] host-transposed inputs; QT/KT [JJ, S] head-dim-major so scores
  come out keys-on-partitions (S.T tiles) and the key-axis softmax reduction
  happens inside the P.T @ V' matmul via a ones-column appended to V'
  (PSUM row 64 of the PV output accumulates the softmax denominator).
  V' stationaries are padded to 128 columns to keep fast weight loads.
  OT [JJ, S] normalized attention output feeds the output projection as
  lhsT, giving the partial output in natural [S, D] layout.
"""
from contextlib import ExitStack

import numpy as np

# Problem constants (hardcoded per harness contract).
B, S, D, H = 2, 2048, 1024, 16
HD = D // H          # 64
N_CORES = 8
GROUPS = N_CORES // B    # 4
H_LOC = H // GROUPS      # 4 heads per core
JJ = H_LOC * HD          # 256
P = 128

MM_DT = "bf16"  # "bf16" | "f32r"


def build_mha(s=S, d=D, h_loc=H_LOC, hd=HD, chunk=1024, nf=512, mm_dt=MM_DT):
    """Build + compile the per-core Bass program."""
    import concourse.bacc as bacc
    import concourse.tile as tile
    from concourse import mybir

    f32 = mybir.dt.float32
    f32r = mybir.dt.float32r
    bf16 = mybir.dt.bfloat16
    mdt = bf16 if mm_dt == "bf16" else f32r
    in_dt = bf16 if mm_dt == "bf16" else f32  # DRAM dtype of x / weights
    Exp = mybir.ActivationFunctionType.Exp
    Ident = mybir.ActivationFunctionType.Identity

    jj = h_loc * hd
    hd1 = hd + 1
    ktd = d // P
    njt = (jj + P - 1) // P
    st_n = s // P
    chunk = min(chunk, s)
    nf = min(nf, chunk)
    n_ch = s // chunk
    nfc = chunk // nf
    ndo = (d + nf - 1) // nf
    pc = min(512, s)

    nc = bacc.Bacc("TRN2", target_bir_lowering=False, debug=False)

    xq = nc.dram_tensor("xq", [d, s], in_dt, kind="ExternalInput").ap()
    xk = nc.dram_tensor("xk", [d, s], in_dt, kind="ExternalInput").ap()
    xv = nc.dram_tensor("xv", [d, s], in_dt, kind="ExternalInput").ap()
    wq = nc.dram_tensor("wq", [d, jj], in_dt, kind="ExternalInput").ap()
    wk = nc.dram_tensor("wk", [d, jj], in_dt, kind="ExternalInput").ap()
    wv = nc.dram_tensor("wv", [d, jj], in_dt, kind="ExternalInput").ap()
    wo = nc.dram_tensor("wo", [jj, d], in_dt, kind="ExternalInput").ap()
    bqp = nc.dram_tensor("bqp", [jj, 1], f32, kind="ExternalInput").ap()
    bkp = nc.dram_tensor("bkp", [jj, 1], f32, kind="ExternalInput").ap()
    out = nc.dram_tensor("out", [s, d], f32, kind="ExternalOutput").ap()

    with tile.TileContext(nc) as tc, ExitStack() as ctx:
        persist = ctx.enter_context(tc.tile_pool(name="persist", bufs=1))

        qt_sb = [persist.tile([P, s], mdt, name=f"qt{j}", tag=f"qt{j}") for j in range(njt)]
        kt_sb = [persist.tile([P, s], mdt, name=f"kt{j}", tag=f"kt{j}") for j in range(njt)]
        ot_sb = [persist.tile([P, s], mdt, name=f"ot{j}", tag=f"ot{j}") for j in range(njt)]
        # padded per-(seq-tile, head) PV stationaries: [V_h | ones | zeros]
        v_sb = [[persist.tile([P, P], mdt, name=f"v{t}_{h}", tag=f"v{t}_{h}")
                 for h in range(h_loc)] for t in range(st_n)]
        wq_r = [persist.tile([P, jj], mdt, name=f"wqr{k}", tag=f"wqr{k}") for k in range(ktd)]
        wk_r = [persist.tile([P, jj], mdt, name=f"wkr{k}", tag=f"wkr{k}") for k in range(ktd)]
        wv_r = [persist.tile([P, jj], mdt, name=f"wvr{k}", tag=f"wvr{k}") for k in range(ktd)]
        wo_r = [persist.tile([P, d], mdt, name=f"wor{j}", tag=f"wor{j}") for j in range(njt)]
        bq_sb = persist.tile([P, njt], f32, name="bq_sb", tag="bq_sb")
        bk_sb = persist.tile([P, njt], f32, name="bk_sb", tag="bk_sb")
        ones_v = persist.tile([P, 1], f32, name="ones_v", tag="ones_v")
        ones_h = persist.tile([1, hd], f32, name="ones_h", tag="ones_h")
        ones_hr = persist.tile([1, hd], f32r, name="ones_hr", tag="ones_hr")

        nc.vector.memset(ones_v[:], 1.0)
        nc.vector.memset(ones_h[:], 1.0)
        nc.vector.tensor_copy(ones_hr[:], ones_h[:])
        for j in range(njt):
            nc.sync.dma_start(bq_sb[:, j:j + 1], bqp[j * P:(j + 1) * P, :])
            nc.sync.dma_start(bk_sb[:, j:j + 1], bkp[j * P:(j + 1) * P, :])

        # ---- weights ----
        if mm_dt == "bf16":
            for k in range(ktd):
                nc.sync.dma_start(wq_r[k][:], wq[k * P:(k + 1) * P, :])
                nc.sync.dma_start(wk_r[k][:], wk[k * P:(k + 1) * P, :])
                nc.sync.dma_start(wv_r[k][:], wv[k * P:(k + 1) * P, :])
            for j in range(njt):
                nc.sync.dma_start(wo_r[j][:], wo[j * P:(j + 1) * P, :])
        else:
            with tc.tile_pool(name="wstage", bufs=3) as wstage:
                for k in range(ktd):
                    for nm, dr, dst in (("q", wq, wq_r), ("k", wk, wk_r), ("v", wv, wv_r)):
                        wtmp = wstage.tile([P, jj], f32, name=f"w{nm}s{k}", tag="wst")
                        nc.sync.dma_start(wtmp[:], dr[k * P:(k + 1) * P, :])
                        nc.vector.tensor_copy(dst[k][:], wtmp[:])
                for j in range(njt):
                    wtmp = wstage.tile([P, d], f32, name=f"wos{j}", tag="wost")
                    nc.sync.dma_start(wtmp[:], wo[j * P:(j + 1) * P, :])
                    nc.vector.tensor_copy(wo_r[j][:], wtmp[:])

        # ---- projections ----
        with tc.tile_pool(name="xpool", bufs=3) as xpool, \
             tc.tile_pool(name="xrpool", bufs=ktd) as xrpool, \
             tc.tile_pool(name="ppsum", bufs=3, space="PSUM") as ppsum:

            def load_xr(xdr):
                tiles = []
                for k in range(ktd):
                    if mm_dt == "bf16":
                        xr = xrpool.tile([P, s], mdt, name=f"xr{k}", tag="xr")
                        nc.sync.dma_start(xr[:], xdr[k * P:(k + 1) * P, :])
                    else:
                        xs = xpool.tile([P, s], f32, name=f"xs{k}", tag="xs")
                        nc.sync.dma_start(xs[:], xdr[k * P:(k + 1) * P, :])
                        xr = xrpool.tile([P, s], mdt, name=f"xr{k}", tag="xr")
                        nc.vector.tensor_copy(xr[:], xs[:])
                    tiles.append(xr)
                return tiles

            for nm, xdr, w_r, dst, bias_sb, scale in (
                ("k", xk, wk_r, kt_sb, bk_sb, 1.0),
                ("q", xq, wq_r, qt_sb, bq_sb, float(1.0 / np.sqrt(hd))),
            ):
                xr_t = load_xr(xdr)
                ncp = s // pc
                for j in range(njt):
                    # k-outer / c-inner so each weight stationary load serves
                    # ncp moving streams
                    pps = [ppsum.tile([P, pc], f32, name=f"pp{nm}{j}_{c}", tag="pp",
                                      bufs=ncp + 1)
                           for c in range(ncp)]
                    for k in range(ktd):
                        for c in range(ncp):
                            nc.tensor.matmul(
                                pps[c][:], w_r[k][:, j * P:(j + 1) * P],
                                xr_t[k][:, c * pc:(c + 1) * pc],
                                start=(k == 0), stop=(k == ktd - 1))
                    for c in range(ncp):
                        nc.scalar.activation(
                            dst[j][:, c * pc:(c + 1) * pc], pps[c][:], Ident,
                            bias=bias_sb[:, j:j + 1], scale=scale)

            # V' padded stationaries
            xr_t = load_xr(xv)
            for t in range(st_n):
                pv = ppsum.tile([P, jj], f32, name=f"pv{t}", tag="pv", bufs=3)
                for k in range(ktd):
                    nc.tensor.matmul(pv[:], xr_t[k][:, t * P:(t + 1) * P],
                                     wv_r[k][:], start=(k == 0), stop=(k == ktd - 1))
                for h in range(h_loc):
                    vt = v_sb[t][h]
                    nc.vector.tensor_copy(vt[:, 0:hd], pv[:, h * hd:(h + 1) * hd])
                    nc.vector.tensor_copy(vt[:, hd:hd1], ones_v[:])
                    if hd1 < P:
                        nc.gpsimd.memset(vt[:, hd1:P], 0.0)

        # ---- attention ----
        # Per head, two passes over the full sequence:
        #   pass 1: scores.T tiles (one KT stationary load per seq-tile, s/nf
        #           moving streams) -> exp over [128, s] -> PT tiles
        #   pass 2: PV accumulation (one V' stationary load per seq-tile,
        #           s/nf moving streams) -> [128, s] psum, row hd = denominators
        # PSUM: sp [128,s] (s/512 banks) + otp [128,s] -> 8 banks total.
        ec = min(1024, s)          # exp / score-psum chunk of the q axis
        nec = s // ec
        efc = ec // nf
        with tc.tile_pool(name="spsum", bufs=2, space="PSUM") as spsum, \
             tc.tile_pool(name="opsum", bufs=1, space="PSUM") as opsum, \
             tc.tile_pool(name="ptpool", bufs=3 * nec + 2) as ptpool, \
             tc.tile_pool(name="npool", bufs=2) as npool:
            for h in range(h_loc):
                jt = (h * hd) // P
                off = (h * hd) % P
                otp = opsum.tile([P, s], f32, name=f"otp{h}", tag="otp")
                pts = {}

                def scores(t):
                    for e in range(nec):
                        sp = spsum.tile([P, ec], f32, name=f"sp{h}_{t}_{e}", tag="sp")
                        for f in range(efc):
                            q0 = e * ec + f * nf
                            nc.tensor.matmul(
                                sp[:, f * nf:(f + 1) * nf],
                                kt_sb[jt][off:off + hd, t * P:(t + 1) * P],
                                qt_sb[jt][off:off + hd, q0:q0 + nf],
                                start=True, stop=True)
                        pt = ptpool.tile([P, ec], mdt, name=f"pt{h}_{t}_{e}", tag="pt")
                        nc.scalar.activation(pt[:], sp[:], Exp)
                        pts[t, e] = pt

                def pv(t):
                    for e in range(nec):
                        for f in range(efc):
                            q0 = e * ec + f * nf
                            nc.tensor.matmul(
                                otp[:, q0:q0 + nf],
                                v_sb[t][h][:],
                                pts[t, e][:, f * nf:(f + 1) * nf],
                                start=(t == 0), stop=(t == st_n - 1))
                        del pts[t, e]

                # software-pipeline: scores(t+1) emitted before pv(t)
                scores(0)
                for t in range(1, st_n):
                    scores(t)
                    pv(t - 1)
                pv(st_n - 1)
                # normalize rows 0:hd by row hd (the softmax denominator)
                rs_r = npool.tile([1, s], f32r, name=f"rs{h}", tag="rs")
                nc.scalar.activation(rs_r[:], otp[hd:hd1, :],
                                     mybir.ActivationFunctionType.Copy)
                for e in range(nec):
                    bp = spsum.tile([hd, ec], f32, name=f"bp{h}_{e}", tag="sp")
                    for f in range(efc):
                        q0 = e * ec + f * nf
                        nc.tensor.matmul(bp[:, f * nf:(f + 1) * nf], ones_hr[:],
                                         rs_r[:, q0:q0 + nf],
                                         start=True, stop=True)
                    binv = npool.tile([hd, ec], f32, name=f"binv{h}_{e}", tag="binv")
                    nc.vector.reciprocal(binv[:], bp[:])
                    nc.vector.tensor_mul(
                        ot_sb[jt][off:off + hd, e * ec:(e + 1) * ec],
                        otp[0:hd, e * ec:(e + 1) * ec], binv[:])

        # ---- output projection (natural layout) ----
        with tc.tile_pool(name="fpsum", bufs=2, space="PSUM") as fpsum, \
             tc.tile_pool(name="fout", bufs=2) as fout:
            for t in range(st_n):
                po = fpsum.tile([P, d], f32, name=f"po{t}", tag="po")
                for njx in range(ndo):
                    for j in range(njt):
                        nc.tensor.matmul(
                            po[:, njx * nf:(njx + 1) * nf],
                            ot_sb[j][:, t * P:(t + 1) * P],
                            wo_r[j][:, njx * nf:(njx + 1) * nf],
                            start=(j == 0), stop=(j == njt - 1))
                ob = fout.tile([P, d], f32, name=f"ob{t}", tag="ob")
                nc.scalar.copy(ob[:], po[:])
                nc.sync.dma_start(out[t * P:(t + 1) * P, :], ob[:])

    nc.compile()
    return nc


_NC_CACHE = {}


def _get_nc():
    key = MM_DT
    if key not in _NC_CACHE:
        _NC_CACHE[key] = build_mha(mm_dt=key)
    return _NC_CACHE[key]


def build_in_maps(inputs, mm_dt=MM_DT):
    import ml_dtypes
    xdt = ml_dtypes.bfloat16 if mm_dt == "bf16" else np.float32

    q = np.asarray(inputs["query"], np.float32)
    k = np.asarray(inputs.get("key_", inputs.get("key")), np.float32)
    v = np.asarray(inputs["value"], np.float32)
    Wq = np.asarray(inputs["Wq"], np.float32)
    Wk = np.asarray(inputs["Wk"], np.float32)
    Wv = np.asarray(inputs["Wv"], np.float32)
    Wo = np.asarray(inputs["Wo"], np.float32)
    bq = np.asarray(inputs["bq"], np.float32)
    bk = np.asarray(inputs["bk"], np.float32)

    sc = np.float32(1.0 / np.sqrt(HD))
    qT = [np.ascontiguousarray(q[b].T).astype(xdt) for b in range(B)]
    kT = [np.ascontiguousarray(k[b].T).astype(xdt) for b in range(B)]
    vT = [np.ascontiguousarray(v[b].T).astype(xdt) for b in range(B)]
    WqT = np.ascontiguousarray(Wq.T)
    WkT = np.ascontiguousarray(Wk.T)
    WvT = np.ascontiguousarray(Wv.T)

    in_maps = []
    for core in range(N_CORES):
        b, g = divmod(core, GROUPS)
        sl = slice(g * JJ, (g + 1) * JJ)
        in_maps.append({
            "xq": qT[b],
            "xk": kT[b],
            "xv": vT[b],
            "wq": np.ascontiguousarray(WqT[:, sl]).astype(xdt),
            "wk": np.ascontiguousarray(WkT[:, sl]).astype(xdt),
            "wv": np.ascontiguousarray(WvT[:, sl]).astype(xdt),
            "wo": np.ascontiguousarray(Wo[:, sl].T).astype(xdt),
            "bqp": np.ascontiguousarray((bq[sl] * sc)[:, None]),
            "bkp": np.ascontiguousarray(bk[sl][:, None]),
        })
    return in_maps


def combine_outputs(results, inputs):
    Wo = np.asarray(inputs["Wo"], np.float32)
    bv = np.asarray(inputs["bv"], np.float32)
    bo = np.asarray(inputs["bo"], np.float32)
    const = bv @ Wo.T + bo  # exact host-side bias correction
    outp = np.empty((B, S, D), np.float32)
    for b in range(B):
        acc = results[b * GROUPS]["out"].astype(np.float32).copy()
        for g in range(1, GROUPS):
            acc += results[b * GROUPS + g]["out"]
        outp[b] = acc + const[None, :]
    return outp


def kernel(**inputs):
    from concourse.bass_utils import run_bass_kernel_spmd

    nc = _get_nc()
    in_maps = build_in_maps(inputs)
    res = run_bass_kernel_spmd(nc, in_maps, list(range(N_CORES)))
    return combine_outputs(res.results, inputs)
